# revision 43
# baseline (speedup 1.0000x reference)
"""Trainium2 Bass kernel for the CRU (complex recurrent unit) problem.

Math: the reference is a T=256-step scan
    pre_t  = x_t @ Wx_big + h_{t-1} @ Wh_big + bias          (B, 2048)
    gate   = sigmoid(pre gate cols), cand = tanh(pre cand cols)
    m      = softmax(x_t @ Wm_ih + h_{t-1} @ Wm_hh + bias_m)  (B, 512)
    h_t    = [m,m] * (gate*h_{t-1} + (1-gate)*cand)

Because m is a softmax over 512 features, |h| ~ 1e-2..1e-3 and the
recurrence is contractive with factor ~5e-3.  A Jacobi fixed-point
iteration over the whole sequence therefore converges in 2 sweeps
(measured: sweep1 rel_err 4.7e-3, sweep2 2.2e-5 in fp32), replacing 256
sequential weight-reload-bound steps with a few large batched matmuls.

Kernel layout (per core, batch-sharded 16 of 128, no collectives):
  feature-major: every tensor is (feature-chunk, 128, token) with
  token n = t*16 + b.  Sweep s computes, for all 4096 tokens at once,
    pre = W.T @ h_prevT + xa      (h_prevT read with a 16-column shift)
  and the gate/cand/softmax elementwise pipeline:
    tau = tanh(0.5*pre_g); u' = (1+tau) h + (1-tau) c
    h   = [e,e] * R * u'  with  e = exp(l - ln2), R = 1/(2*sum e)
  (the 0.5's from sigmoid(x)=0.5+0.5 tanh(x/2) are folded into the
  exp bias and the "2.0" weights of the column-sum matmul).
"""

import numpy as np
import ml_dtypes

import concourse.bass as bass
import concourse.bacc as bacc
import concourse.mybir as mybir
import concourse.tile as tile
from concourse.bass_utils import run_bass_kernel_spmd

BF16 = mybir.dt.bfloat16
F32 = mybir.dt.float32
FP8 = mybir.dt.float8e4
AF = mybir.ActivationFunctionType
ALU = mybir.AluOpType

N_CORES = 8
T, B, I, H = 256, 128, 512, 512
BL = B // N_CORES          # batch per core
NTOK = T * BL              # 4096 tokens per core
KC = 8                     # contraction chunks of 128 (K = 1024)
MC = 20                    # output-feature chunks of 128 (M = 2560)
GC, CC, EC = 8, 8, 4       # gate / cand / mod chunks within MC
TC = 8                     # token chunks of 512
TW = NTOK // TC            # 512 tokens per chunk
N_SWEEPS = 2
LN2 = float(np.log(2.0))
SW = 1024.0                # fp8 weight scale (h-sweep)
SH = 64.0                  # fp8 hidden-state scale

_NC_CACHE = {}


def _chunk_bcast(ap, nrep):
    """View a (128, TW) AP as (128, nrep, TW) with a stride-0 repeat dim."""
    return bass.AP(ap.tensor, ap.offset, [ap.ap[0], [0, nrep], ap.ap[1]])


def _build_phase(nc, tc, pools, w_sb, bias_sb, consts, first, rhs_dram, xa_dram,
                 dst_dram, dst_off, dst_fp8):
    """One Jacobi sweep.  first=True fuses the x-matmul with sweep 1
    (h_prev treated as 0 everywhere; the dropped hx@Wh term for t=0 is
    corrected by sweep 2, which reads hx from hg columns 0:BL).
    Non-first sweeps consume fp8 h (x SH) and fp8 weights (x SW) with
    DoubleRow pairing of adjacent k-chunks; descale rides the pre-STT."""
    rhsp, xap, actp, ewp, outp, pp, ppr, ppb = pools
    twos, ones_row, negln2, ident = consts
    n_acc = KC if first else KC // 2
    # non-first sweeps: psum holds 65536*pre (identity preload of xa is
    # scaled by SW*SH = 65536 to match the fp8 weight/state scales), so
    # the descale rides the ACT input affine for free.
    asc = 1.0 if first else 1.0 / (SW * SH)

    for tch in range(TC):
        tsl = slice(tch * TW, (tch + 1) * TW)
        # all 8 k-chunks in one DMA; doubles as h_prev for the elementwise
        hrt = rhsp.tile([128, KC, TW], BF16 if first else FP8, tag="rhs",
                        bufs=2, name="hrt")
        nc.sync.dma_start(hrt[:], rhs_dram[:, :, tsl].rearrange("k p n -> p k n"))
        if first:
            h_ew = hrt
        else:
            # descaled bf16 copy of h for the elementwise chain
            h_ew = rhsp.tile([128, KC, TW], BF16, tag="hbf", bufs=1, name="h_ew")
            nc.scalar.activation(h_ew[:], hrt[:], AF.Copy, scale=1.0 / SH)

        tau_t = actp.tile([128, GC, TW], BF16, tag="tau", name="tau_t")
        c_t = actp.tile([128, CC, TW], BF16, tag="cand", name="c_t")
        e_t = actp.tile([128, EC, TW], BF16, tag="e", name="e_t")

        # First tchunk runs k-outer over m-groups so the PE can start as
        # soon as weight chunk k=0 arrives instead of waiting for all 8.
        if tch == 0:
            m_plan = [list(range(g, min(g + 4, MC))) for g in range(0, MC, 4)]
        else:
            m_plan = [[m] for m in range(MC)]

        for mg in m_plan:
            pss = [pp.tile([128, TW], F32, tag="ps", name=f"ps{m}") for m in mg]
            xa_tiles = {}
            if not first:
                # preload psum with 65536*xa via a scaled-identity matmul
                for mi, m in enumerate(mg):
                    xa_sb = xap.tile([128, TW], BF16, tag="xa", bufs=6,
                                     name="xa_sb")
                    nc.sync.dma_start(xa_sb[:], xa_dram[m, :, tsl])
                    xa_tiles[m] = xa_sb
                for mi, m in enumerate(mg):
                    nc.tensor.matmul(pss[mi][:], ident[:], xa_tiles[m][:],
                                     start=True, stop=False)
            for k in range(n_acc):
                for mi, m in enumerate(mg):
                    if first:
                        nc.tensor.matmul(
                            pss[mi][:],
                            w_sb[k][:, m * 128:(m + 1) * 128],
                            hrt[:, k, :],
                            start=(k == 0),
                            stop=(k == n_acc - 1),
                        )
                    else:
                        nc.tensor.matmul(
                            pss[mi][:],
                            w_sb[k][:, :, m * 128:(m + 1) * 128],
                            hrt[:, 2 * k:2 * k + 2, :],
                            start=False,
                            stop=(k == n_acc - 1),
                            perf_mode=mybir.MatmulPerfMode.DoubleRow,
                        )
            for mi, m in enumerate(mg):
                ps = pss[mi]
                if first:
                    # pre = psum + bias; alternate ACT/DVE to balance engines
                    pre = xap.tile([128, TW], BF16, tag="xa", bufs=6,
                                   name="xa_sb")
                    if m % 2 == 0:
                        nc.scalar.add(pre[:], ps[:], bias_sb[:, m:m + 1])
                    else:
                        nc.vector.tensor_scalar_add(pre[:], ps[:],
                                                    bias_sb[:, m:m + 1])
                    nc.sync.dma_start(xa_dram[m, :, tsl], pre[:])
                    src = pre
                else:
                    src = ps  # activations descale the psum via `asc`
                # activations (tanh & exp share one ACT table set)
                if m < GC:
                    nc.scalar.activation(tau_t[:, m, :], src[:], AF.Tanh,
                                         scale=0.5 * asc)
                elif m < GC + CC:
                    nc.scalar.activation(c_t[:, m - GC, :], src[:], AF.Tanh,
                                         scale=asc)
                else:
                    nc.scalar.activation(e_t[:, m - GC - CC, :], src[:],
                                         AF.Exp, bias=negln2[:], scale=asc)

        # R = 1 / (2 * sum_feat e), broadcast to 128 partitions via PE
        s1 = ewp.tile([128, TW], BF16, tag="ssum", bufs=2, name="s1")
        s2 = ewp.tile([128, TW], BF16, tag="ssum2", bufs=2, name="s2")
        nc.vector.tensor_add(s1[:], e_t[:, 0, :], e_t[:, 1, :])
        nc.vector.tensor_add(s2[:], e_t[:, 2, :], e_t[:, 3, :])
        s3 = ewp.tile([128, TW], BF16, tag="ssum3", bufs=2, name="s3")
        nc.vector.tensor_add(s3[:], s1[:], s2[:])
        psr = ppr.tile([1, TW], F32, tag="psr", name="psr")
        nc.tensor.matmul(psr[:], twos[:], s3[:], start=True, stop=True)
        rv = ewp.tile([1, TW], F32, tag="rv", bufs=2, name="rv")
        nc.vector.reciprocal_approx_fast(rv[:], psr[:])
        psb = ppb.tile([128, TW], F32, tag="psb", name="psb")
        nc.tensor.matmul(psb[:], ones_row[:], rv[:], start=True, stop=True)
        rb = ewp.tile([128, TW], BF16, tag="rb", bufs=2, name="rb")
        nc.scalar.copy(rb[:], psb[:])

        # w = e * R  (one wide op, R repeated across the 4 chunks)
        w_t = ewp.tile([128, EC, TW], BF16, tag="wm", bufs=2, name="w_t")
        nc.vector.tensor_tensor(w_t[:], e_t[:], _chunk_bcast(rb[:], EC), ALU.mult)

        # u' = (1+tau) h + (1-tau) c  as wide plain-TT ops (2x DVE mode)
        if first:
            p0 = ewp.tile([128, KC, TW], BF16, tag="up", bufs=1, name="p0")
            nc.vector.tensor_tensor(p0[:], tau_t[:], c_t[:], ALU.mult)
            ut = ewp.tile([128, KC, TW], BF16, tag="uu", bufs=1, name="ut")
            nc.vector.tensor_tensor(ut[:], c_t[:], p0[:], ALU.subtract)
        else:
            at = ewp.tile([128, KC, TW], BF16, tag="ua", bufs=1, name="at")
            nc.vector.tensor_tensor(at[:], h_ew[:], c_t[:], ALU.add)
            dt_ = ewp.tile([128, KC, TW], BF16, tag="ud", bufs=1, name="dt_")
            nc.vector.tensor_tensor(dt_[:], h_ew[:], c_t[:], ALU.subtract)
            p0 = ewp.tile([128, KC, TW], BF16, tag="up", bufs=1, name="p0")
            nc.vector.tensor_tensor(p0[:], tau_t[:], dt_[:], ALU.mult)
            ut = ewp.tile([128, KC, TW], BF16, tag="uu", bufs=1, name="ut")
            nc.vector.tensor_tensor(ut[:], at[:], p0[:], ALU.add)

        # h_new = [w,w] * u'   (two half-wide ops reusing w_t)
        hn = outp.tile([128, KC, TW], BF16, tag="hn", bufs=2, name="hn")
        nc.vector.tensor_tensor(hn[:, 0:EC, :], w_t[:], ut[:, 0:EC, :], ALU.mult)
        nc.vector.tensor_tensor(hn[:, EC:KC, :], w_t[:], ut[:, EC:KC, :], ALU.mult)
        if dst_fp8:
            hn8 = outp.tile([128, KC, TW], FP8, tag="hn8", bufs=1, name="hn8")
            nc.scalar.activation(hn8[:], hn[:], AF.Copy, scale=SH)
            hn = hn8
        nc.sync.dma_start(
            dst_dram[:, :, dst_off + tch * TW: dst_off + (tch + 1) * TW]
            .rearrange("k p n -> p k n"),
            hn[:])


def _build_nc():
    nc = bacc.Bacc("TRN2", target_bir_lowering=False)

    xT = nc.dram_tensor("xT", (KC, 128, NTOK), BF16, kind="ExternalInput")
    hxT = nc.dram_tensor("hxT", (KC, 128, BL), BF16, kind="ExternalInput")
    wx = nc.dram_tensor("wx", (KC, 128, MC * 128), BF16, kind="ExternalInput")
    wh = nc.dram_tensor("wh", (KC // 2, 128, 2, MC * 128), FP8,
                        kind="ExternalInput")
    bias = nc.dram_tensor("bias", (128, MC), F32, kind="ExternalInput")
    ident_d = nc.dram_tensor("ident", (128, 128), BF16, kind="ExternalInput")
    out = nc.dram_tensor("out", (KC, 128, NTOK), BF16, kind="ExternalOutput")

    xa_d = nc.dram_tensor("xa_scratch", (MC, 128, NTOK), BF16)
    hg = [nc.dram_tensor(f"hg_{i}", (KC, 128, BL + NTOK), FP8)
          for i in range(N_SWEEPS - 1)]

    with tile.TileContext(nc) as tc:
        with (
            tc.tile_pool(name="weights", bufs=2) as wp,
            tc.tile_pool(name="const", bufs=1) as cp,
            tc.tile_pool(name="rhs", bufs=2) as rhsp,
            tc.tile_pool(name="xa", bufs=6) as xap,
            tc.tile_pool(name="acts", bufs=2) as actp,
            tc.tile_pool(name="ew", bufs=2) as ewp,
            tc.tile_pool(name="hout", bufs=2) as outp,
            tc.tile_pool(name="ps", bufs=6, space=bass.MemorySpace.PSUM) as pp,
            tc.tile_pool(name="psr", bufs=1, space=bass.MemorySpace.PSUM) as ppr,
            tc.tile_pool(name="psb", bufs=1, space=bass.MemorySpace.PSUM) as ppb,
        ):
            pools = (rhsp, xap, actp, ewp, outp, pp, ppr, ppb)

            bias_sb = cp.tile([128, MC], F32, tag="bias", name="bias_sb")
            nc.sync.dma_start(bias_sb[:], bias[:])
            twos = cp.tile([128, 1], BF16, tag="twos", name="twos")
            nc.vector.memset(twos[:], 2.0)
            ones_row = cp.tile([1, 128], F32, tag="ones_row", name="ones_row")
            nc.vector.memset(ones_row[:], 1.0)
            negln2 = cp.tile([128, 1], F32, tag="negln2", name="negln2")
            nc.vector.memset(negln2[:], -LN2)
            ident = cp.tile([128, 128], BF16, tag="ident", name="ident")
            nc.sync.dma_start(ident[:], ident_d[:])
            consts = (twos, ones_row, negln2, ident)

            # hx -> first BL columns of hg_0 (via SBUF, scaled to fp8 state)
            hx_sb = cp.tile([128, KC, BL], BF16, tag="hx", name="hx_sb")
            nc.sync.dma_start(hx_sb[:], hxT.rearrange("k p b -> p k b"))
            hx8 = cp.tile([128, KC, BL], FP8, tag="hx8", name="hx8")
            nc.scalar.activation(hx8[:], hx_sb[:], AF.Copy, scale=SH)
            nc.sync.dma_start(hg[0][:, :, 0:BL].rearrange("k p b -> p k b"),
                              hx8[:])

            # sweep 1 (fused x-matmul), writes xa + hg_0 shifted by BL
            # weight DMAs split into column-quarters, issued quarter-major,
            # so the first m-groups' matmuls start ~1us in instead of
            # waiting for the full 5.24MB load.
            QW = MC * 128 // 4
            wx_sb = [wp.tile([128, MC * 128], BF16, tag=f"w{k}", bufs=1,
                             name=f"wx{k}") for k in range(KC)]
            for q in range(4):
                for k in range(KC):
                    nc.sync.dma_start(wx_sb[k][:, q * QW:(q + 1) * QW],
                                      wx[k, :, q * QW:(q + 1) * QW])
            _build_phase(nc, tc, pools, wx_sb, bias_sb, consts, True,
                         xT, xa_d, hg[0], BL, True)

            # sweeps 2..N (fp8 DoubleRow: 4 tiles pairing adjacent k-chunks)
            wh_sb = [wp.tile([128, 2, MC * 128], FP8, tag=f"w{j}", bufs=1,
                             name=f"wh{j}") for j in range(KC // 2)]
            for q in range(4):
                for j in range(KC // 2):
                    nc.sync.dma_start(wh_sb[j][:, :, q * QW:(q + 1) * QW],
                                      wh[j, :, :, q * QW:(q + 1) * QW])
            for s in range(1, N_SWEEPS):
                last = s == N_SWEEPS - 1
                dst = out if last else hg[s]
                _build_phase(nc, tc, pools, wh_sb, bias_sb, consts, False,
                             hg[s - 1], xa_d, dst, 0 if last else BL, not last)

    nc.compile()
    return nc


def _get_nc():
    if "nc" not in _NC_CACHE:
        _NC_CACHE["nc"] = _build_nc()
    return _NC_CACHE["nc"]


def _prep_inputs(inputs):
    f = lambda k: np.asarray(inputs[k], np.float32)
    x, hx = f("x"), f("hx")
    Wr_ih, Wi_ih = f("w_ih_real"), f("w_ih_imag")
    Wr_hh, Wi_hh = f("w_hh_real"), f("w_hh_imag")
    b_ih, b_hh = f("b_ih"), f("b_hh")
    Wm_ih, Wm_hh = f("w_ih_mod"), f("w_hh_mod")
    bm_ih, bm_hh = f("b_ih_mod"), f("b_hh_mod")

    def big(Wr, Wi):
        return np.block([[Wr, Wi], [-Wi, Wr]])

    perm = np.concatenate([
        np.arange(0, 512), np.arange(1024, 1536),       # gate: [gr | gi]
        np.arange(512, 1024), np.arange(1536, 2048),    # cand: [cr | ci]
    ])
    W_x = np.concatenate([big(Wr_ih, Wi_ih)[:, perm], Wm_ih], axis=1)
    W_h = np.concatenate([big(Wr_hh, Wi_hh)[:, perm], Wm_hh], axis=1)
    bias_all = np.concatenate([(b_ih + b_hh)[perm], bm_ih + bm_hh])

    bf = ml_dtypes.bfloat16
    wx_in = np.ascontiguousarray(W_x.reshape(KC, 128, MC * 128)).astype(bf)
    # fp8 h-weights: scale by SW, pair adjacent k-chunks for DoubleRow:
    # [pair j, partition p, ko, m] = W_h[(2j+ko)*128 + p, m] * SW
    wh8 = np.clip(W_h * SW, -448.0, 448.0)
    wh_in = np.ascontiguousarray(
        wh8.reshape(KC // 2, 2, 128, MC * 128).transpose(0, 2, 1, 3)
    ).astype(ml_dtypes.float8_e4m3)
    bias_in = np.ascontiguousarray(bias_all.reshape(MC, 128).T).astype(np.float32)

    in_maps = []
    for c in range(N_CORES):
        bs = slice(c * BL, (c + 1) * BL)
        xs = x[:, bs, :]                                   # (T, BL, 1024)
        xT_c = np.ascontiguousarray(
            xs.transpose(2, 0, 1).reshape(KC, 128, NTOK)).astype(bf)
        hxT_c = np.ascontiguousarray(
            hx[bs].T.reshape(KC, 128, BL)).astype(bf)
        in_maps.append({
            "xT": xT_c, "hxT": hxT_c,
            "wx": wx_in, "wh": wh_in, "bias": bias_in,
            "ident": (np.eye(128, dtype=np.float32) * (SW * SH)
                      ).astype(ml_dtypes.bfloat16),
        })
    return in_maps


def run(inputs, trace=False):
    nc = _get_nc()
    in_maps = _prep_inputs(inputs)
    res = run_bass_kernel_spmd(nc, in_maps, core_ids=list(range(N_CORES)),
                               trace=trace)
    outputs = np.empty((T, B, 2 * H), np.float32)
    for c in range(N_CORES):
        o = res.results[c]["out"].astype(np.float32).reshape(2 * H, T, BL)
        outputs[:, c * BL:(c + 1) * BL, :] = o.transpose(1, 2, 0)
    return (outputs, outputs[-1].copy()), res.exec_time_ns


def kernel(**inputs):
    return run(inputs, trace=False)[0]


# revision 44
# speedup vs baseline: 1.0198x; 1.0198x over previous
"""Trainium2 Bass kernel for the CRU (complex recurrent unit) problem.

Math: the reference is a T=256-step scan
    pre_t  = x_t @ Wx_big + h_{t-1} @ Wh_big + bias          (B, 2048)
    gate   = sigmoid(pre gate cols), cand = tanh(pre cand cols)
    m      = softmax(x_t @ Wm_ih + h_{t-1} @ Wm_hh + bias_m)  (B, 512)
    h_t    = [m,m] * (gate*h_{t-1} + (1-gate)*cand)

Because m is a softmax over 512 features, |h| ~ 1e-2..1e-3 and the
recurrence is contractive with factor ~5e-3.  A Jacobi fixed-point
iteration over the whole sequence therefore converges in 2 sweeps
(measured: sweep1 rel_err 4.7e-3, sweep2 2.2e-5 in fp32), replacing 256
sequential weight-reload-bound steps with a few large batched matmuls.

Kernel layout (per core, batch-sharded 16 of 128, no collectives):
  feature-major: every tensor is (feature-chunk, 128, token) with
  token n = t*16 + b.  Sweep s computes, for all 4096 tokens at once,
    pre = W.T @ h_prevT + xa      (h_prevT read with a 16-column shift)
  and the gate/cand/softmax elementwise pipeline:
    tau = tanh(0.5*pre_g); u' = (1+tau) h + (1-tau) c
    h   = [e,e] * R * u'  with  e = exp(l - ln2), R = 1/(2*sum e)
  (the 0.5's from sigmoid(x)=0.5+0.5 tanh(x/2) are folded into the
  exp bias and the "2.0" weights of the column-sum matmul).
"""

import numpy as np
import ml_dtypes

import concourse.bass as bass
import concourse.bacc as bacc
import concourse.mybir as mybir
import concourse.tile as tile
from concourse.bass_utils import run_bass_kernel_spmd

BF16 = mybir.dt.bfloat16
F32 = mybir.dt.float32
FP8 = mybir.dt.float8e4
AF = mybir.ActivationFunctionType
ALU = mybir.AluOpType

N_CORES = 8
T, B, I, H = 256, 128, 512, 512
BL = B // N_CORES          # batch per core
NTOK = T * BL              # 4096 tokens per core
KC = 8                     # contraction chunks of 128 (K = 1024)
MC = 20                    # output-feature chunks of 128 (M = 2560)
GC, CC, EC = 8, 8, 4       # gate / cand / mod chunks within MC
TC = 8                     # token chunks of 512
TW = NTOK // TC            # 512 tokens per chunk
N_SWEEPS = 2
LN2 = float(np.log(2.0))
SW = 1024.0                # fp8 weight scale (h-sweep)
SH = 64.0                  # fp8 hidden-state scale

_NC_CACHE = {}


def _chunk_bcast(ap, nrep):
    """View a (128, TW) AP as (128, nrep, TW) with a stride-0 repeat dim."""
    return bass.AP(ap.tensor, ap.offset, [ap.ap[0], [0, nrep], ap.ap[1]])


def _build_phase(nc, tc, pools, w_sb, bias_sb, consts, first, rhs_dram, xa_dram,
                 dst_dram, dst_off, dst_fp8):
    """One Jacobi sweep.  first=True fuses the x-matmul with sweep 1
    (h_prev treated as 0 everywhere; the dropped hx@Wh term for t=0 is
    corrected by sweep 2, which reads hx from hg columns 0:BL).
    Non-first sweeps consume fp8 h (x SH) and fp8 weights (x SW) with
    DoubleRow pairing of adjacent k-chunks; descale rides the pre-STT."""
    rhsp, xap, actp, ewp, outp, pp, ppr, ppb = pools
    twos, ones_row, negln2, ident = consts
    n_acc = KC if first else KC // 2
    # non-first sweeps: psum holds 65536*pre (identity preload of xa is
    # scaled by SW*SH = 65536 to match the fp8 weight/state scales), so
    # the descale rides the ACT input affine for free.
    asc = 1.0 if first else 1.0 / (SW * SH)

    for tch in range(TC):
        tsl = slice(tch * TW, (tch + 1) * TW)
        # all 8 k-chunks in one DMA; doubles as h_prev for the elementwise
        hrt = rhsp.tile([128, KC, TW], BF16 if first else FP8, tag="rhs",
                        bufs=2, name="hrt")
        nc.sync.dma_start(hrt[:], rhs_dram[:, :, tsl].rearrange("k p n -> p k n"))
        if first:
            h_ew = hrt
        else:
            # descaled bf16 copy of h for the elementwise chain
            h_ew = rhsp.tile([128, KC, TW], BF16, tag="hbf", bufs=1, name="h_ew")
            nc.scalar.activation(h_ew[:], hrt[:], AF.Copy, scale=1.0 / SH)

        tau_t = actp.tile([128, GC, TW], BF16, tag="tau", name="tau_t")
        c_t = actp.tile([128, CC, TW], BF16, tag="cand", name="c_t")
        e_t = actp.tile([128, EC, TW], BF16, tag="e", name="e_t")

        # First tchunk runs k-outer over m-groups so the PE can start as
        # soon as weight chunk k=0 arrives instead of waiting for all 8.
        if tch == 0:
            m_plan = [list(range(g, min(g + 4, MC))) for g in range(0, MC, 4)]
        else:
            m_plan = [[m] for m in range(MC)]

        for mg in m_plan:
            pss = [pp.tile([128, TW], F32, tag="ps", name=f"ps{m}") for m in mg]
            xa_tiles = {}
            if not first:
                # preload psum with 65536*xa via a scaled-identity matmul
                for mi, m in enumerate(mg):
                    xa_sb = xap.tile([128, TW], BF16, tag="xa", bufs=6,
                                     name="xa_sb")
                    nc.sync.dma_start(xa_sb[:], xa_dram[m, :, tsl])
                    xa_tiles[m] = xa_sb
                for mi, m in enumerate(mg):
                    nc.tensor.matmul(pss[mi][:], ident[:], xa_tiles[m][:],
                                     start=True, stop=False)
            for k in range(n_acc):
                for mi, m in enumerate(mg):
                    if first:
                        nc.tensor.matmul(
                            pss[mi][:],
                            w_sb[k][:, m * 128:(m + 1) * 128],
                            hrt[:, k, :],
                            start=(k == 0),
                            stop=(k == n_acc - 1),
                        )
                    else:
                        nc.tensor.matmul(
                            pss[mi][:],
                            w_sb[k][:, :, m * 128:(m + 1) * 128],
                            hrt[:, 2 * k:2 * k + 2, :],
                            start=False,
                            stop=(k == n_acc - 1),
                            perf_mode=mybir.MatmulPerfMode.DoubleRow,
                        )
            for mi, m in enumerate(mg):
                ps = pss[mi]
                if first:
                    # pre = psum + bias; alternate ACT/DVE to balance engines
                    pre = xap.tile([128, TW], BF16, tag="xa", bufs=6,
                                   name="xa_sb")
                    if m % 2 == 0:
                        nc.scalar.add(pre[:], ps[:], bias_sb[:, m:m + 1])
                    else:
                        nc.vector.tensor_scalar_add(pre[:], ps[:],
                                                    bias_sb[:, m:m + 1])
                    nc.sync.dma_start(xa_dram[m, :, tsl], pre[:])
                    src = pre
                else:
                    src = ps  # activations descale the psum via `asc`
                # activations (tanh & exp share one ACT table set)
                if m < GC:
                    nc.scalar.activation(tau_t[:, m, :], src[:], AF.Tanh,
                                         scale=0.5 * asc)
                elif m < GC + CC:
                    nc.scalar.activation(c_t[:, m - GC, :], src[:], AF.Tanh,
                                         scale=asc)
                else:
                    nc.scalar.activation(e_t[:, m - GC - CC, :], src[:],
                                         AF.Exp, bias=negln2[:], scale=asc)

        # R = 1 / (2 * sum_feat e), broadcast to 128 partitions via PE
        s1 = ewp.tile([128, TW], BF16, tag="ssum", bufs=2, name="s1")
        s2 = ewp.tile([128, TW], BF16, tag="ssum2", bufs=2, name="s2")
        nc.vector.tensor_add(s1[:], e_t[:, 0, :], e_t[:, 1, :])
        nc.vector.tensor_add(s2[:], e_t[:, 2, :], e_t[:, 3, :])
        s3 = ewp.tile([128, TW], BF16, tag="ssum3", bufs=2, name="s3")
        nc.vector.tensor_add(s3[:], s1[:], s2[:])
        psr = ppr.tile([1, TW], F32, tag="psr", name="psr")
        nc.tensor.matmul(psr[:], twos[:], s3[:], start=True, stop=True)
        rv = ewp.tile([1, TW], F32, tag="rv", bufs=2, name="rv")
        nc.vector.reciprocal_approx_fast(rv[:], psr[:])
        psb = ppb.tile([128, TW], F32, tag="psb", name="psb")
        nc.tensor.matmul(psb[:], ones_row[:], rv[:], start=True, stop=True)
        rb = ewp.tile([128, TW], BF16, tag="rb", bufs=2, name="rb")
        nc.scalar.copy(rb[:], psb[:])

        # w = e * R  (one wide op, R repeated across the 4 chunks)
        w_t = ewp.tile([128, EC, TW], BF16, tag="wm", bufs=2, name="w_t")
        nc.vector.tensor_tensor(w_t[:], e_t[:], _chunk_bcast(rb[:], EC), ALU.mult)

        # u' = (1+tau) h + (1-tau) c  as wide plain-TT ops (2x DVE mode)
        if first:
            p0 = ewp.tile([128, KC, TW], BF16, tag="up", bufs=1, name="p0")
            nc.vector.tensor_tensor(p0[:], tau_t[:], c_t[:], ALU.mult)
            ut = ewp.tile([128, KC, TW], BF16, tag="uu", bufs=1, name="ut")
            nc.vector.tensor_tensor(ut[:], c_t[:], p0[:], ALU.subtract)
        else:
            at = ewp.tile([128, KC, TW], BF16, tag="ua", bufs=1, name="at")
            nc.vector.tensor_tensor(at[:], h_ew[:], c_t[:], ALU.add)
            dt_ = ewp.tile([128, KC, TW], BF16, tag="ud", bufs=1, name="dt_")
            nc.vector.tensor_tensor(dt_[:], h_ew[:], c_t[:], ALU.subtract)
            p0 = ewp.tile([128, KC, TW], BF16, tag="up", bufs=1, name="p0")
            nc.vector.tensor_tensor(p0[:], tau_t[:], dt_[:], ALU.mult)
            ut = ewp.tile([128, KC, TW], BF16, tag="uu", bufs=1, name="ut")
            nc.vector.tensor_tensor(ut[:], at[:], p0[:], ALU.add)

        # h_new = [w,w] * u'   (two half-wide ops reusing w_t)
        hn = outp.tile([128, KC, TW], BF16, tag="hn", bufs=2, name="hn")
        nc.vector.tensor_tensor(hn[:, 0:EC, :], w_t[:], ut[:, 0:EC, :], ALU.mult)
        nc.vector.tensor_tensor(hn[:, EC:KC, :], w_t[:], ut[:, EC:KC, :], ALU.mult)
        if dst_fp8:
            hn8 = outp.tile([128, KC, TW], FP8, tag="hn8", bufs=1, name="hn8")
            nc.scalar.activation(hn8[:], hn[:], AF.Copy, scale=SH)
            hn = hn8
        nc.sync.dma_start(
            dst_dram[:, :, dst_off + tch * TW: dst_off + (tch + 1) * TW]
            .rearrange("k p n -> p k n"),
            hn[:])


def _build_nc():
    nc = bacc.Bacc("TRN2", target_bir_lowering=False)

    xT = nc.dram_tensor("xT", (KC, 128, NTOK), BF16, kind="ExternalInput")
    hxT = nc.dram_tensor("hxT", (KC, 128, BL), BF16, kind="ExternalInput")
    wx = nc.dram_tensor("wx", (KC, 128, MC * 128), BF16, kind="ExternalInput")
    wh = nc.dram_tensor("wh", (KC // 2, 128, 2, MC * 128), FP8,
                        kind="ExternalInput")
    bias = nc.dram_tensor("bias", (128, MC), F32, kind="ExternalInput")
    ident_d = nc.dram_tensor("ident", (128, 128), BF16, kind="ExternalInput")
    out = nc.dram_tensor("out", (KC, 128, NTOK), BF16, kind="ExternalOutput")

    xa_d = nc.dram_tensor("xa_scratch", (MC, 128, NTOK), BF16)
    hg = [nc.dram_tensor(f"hg_{i}", (KC, 128, BL + NTOK), FP8)
          for i in range(N_SWEEPS - 1)]

    with tile.TileContext(nc) as tc:
        with (
            tc.tile_pool(name="weights", bufs=2) as wp,
            tc.tile_pool(name="const", bufs=1) as cp,
            tc.tile_pool(name="rhs", bufs=2) as rhsp,
            tc.tile_pool(name="xa", bufs=6) as xap,
            tc.tile_pool(name="acts", bufs=2) as actp,
            tc.tile_pool(name="ew", bufs=2) as ewp,
            tc.tile_pool(name="hout", bufs=2) as outp,
            tc.tile_pool(name="ps", bufs=6, space=bass.MemorySpace.PSUM) as pp,
            tc.tile_pool(name="psr", bufs=1, space=bass.MemorySpace.PSUM) as ppr,
            tc.tile_pool(name="psb", bufs=1, space=bass.MemorySpace.PSUM) as ppb,
        ):
            pools = (rhsp, xap, actp, ewp, outp, pp, ppr, ppb)

            bias_sb = cp.tile([128, MC], F32, tag="bias", name="bias_sb")
            nc.sync.dma_start(bias_sb[:], bias[:])
            twos = cp.tile([128, 1], BF16, tag="twos", name="twos")
            nc.vector.memset(twos[:], 2.0)
            ones_row = cp.tile([1, 128], F32, tag="ones_row", name="ones_row")
            nc.vector.memset(ones_row[:], 1.0)
            negln2 = cp.tile([128, 1], F32, tag="negln2", name="negln2")
            nc.vector.memset(negln2[:], -LN2)
            ident = cp.tile([128, 128], BF16, tag="ident", name="ident")
            nc.sync.dma_start(ident[:], ident_d[:])
            consts = (twos, ones_row, negln2, ident)

            # hx -> first BL columns of hg_0 (via SBUF, scaled to fp8 state)
            hx_sb = cp.tile([128, KC, BL], BF16, tag="hx", name="hx_sb")
            nc.sync.dma_start(hx_sb[:], hxT.rearrange("k p b -> p k b"))
            hx8 = cp.tile([128, KC, BL], FP8, tag="hx8", name="hx8")
            nc.scalar.activation(hx8[:], hx_sb[:], AF.Copy, scale=SH)
            nc.sync.dma_start(hg[0][:, :, 0:BL].rearrange("k p b -> p k b"),
                              hx8[:])

            # sweep 1 (fused x-matmul), writes xa + hg_0 shifted by BL
            wx_sb = [wp.tile([128, MC * 128], BF16, tag=f"w{k}", bufs=1,
                             name=f"wx{k}") for k in range(KC)]
            for k in range(KC):
                nc.sync.dma_start(wx_sb[k][:], wx[k])
            _build_phase(nc, tc, pools, wx_sb, bias_sb, consts, True,
                         xT, xa_d, hg[0], BL, True)

            # sweeps 2..N (fp8 DoubleRow: 4 tiles pairing adjacent k-chunks)
            wh_sb = [wp.tile([128, 2, MC * 128], FP8, tag=f"w{j}", bufs=1,
                             name=f"wh{j}") for j in range(KC // 2)]
            for j in range(KC // 2):
                nc.sync.dma_start(wh_sb[j][:], wh[j])
            for s in range(1, N_SWEEPS):
                last = s == N_SWEEPS - 1
                dst = out if last else hg[s]
                _build_phase(nc, tc, pools, wh_sb, bias_sb, consts, False,
                             hg[s - 1], xa_d, dst, 0 if last else BL, not last)

    nc.compile()
    return nc


def _get_nc():
    if "nc" not in _NC_CACHE:
        _NC_CACHE["nc"] = _build_nc()
    return _NC_CACHE["nc"]


def _prep_inputs(inputs):
    f = lambda k: np.asarray(inputs[k], np.float32)
    x, hx = f("x"), f("hx")
    Wr_ih, Wi_ih = f("w_ih_real"), f("w_ih_imag")
    Wr_hh, Wi_hh = f("w_hh_real"), f("w_hh_imag")
    b_ih, b_hh = f("b_ih"), f("b_hh")
    Wm_ih, Wm_hh = f("w_ih_mod"), f("w_hh_mod")
    bm_ih, bm_hh = f("b_ih_mod"), f("b_hh_mod")

    def big(Wr, Wi):
        return np.block([[Wr, Wi], [-Wi, Wr]])

    perm = np.concatenate([
        np.arange(0, 512), np.arange(1024, 1536),       # gate: [gr | gi]
        np.arange(512, 1024), np.arange(1536, 2048),    # cand: [cr | ci]
    ])
    W_x = np.concatenate([big(Wr_ih, Wi_ih)[:, perm], Wm_ih], axis=1)
    W_h = np.concatenate([big(Wr_hh, Wi_hh)[:, perm], Wm_hh], axis=1)
    bias_all = np.concatenate([(b_ih + b_hh)[perm], bm_ih + bm_hh])

    bf = ml_dtypes.bfloat16
    wx_in = np.ascontiguousarray(W_x.reshape(KC, 128, MC * 128)).astype(bf)
    # fp8 h-weights: scale by SW, pair adjacent k-chunks for DoubleRow:
    # [pair j, partition p, ko, m] = W_h[(2j+ko)*128 + p, m] * SW
    wh8 = np.clip(W_h * SW, -448.0, 448.0)
    wh_in = np.ascontiguousarray(
        wh8.reshape(KC // 2, 2, 128, MC * 128).transpose(0, 2, 1, 3)
    ).astype(ml_dtypes.float8_e4m3)
    bias_in = np.ascontiguousarray(bias_all.reshape(MC, 128).T).astype(np.float32)

    in_maps = []
    for c in range(N_CORES):
        bs = slice(c * BL, (c + 1) * BL)
        xs = x[:, bs, :]                                   # (T, BL, 1024)
        xT_c = np.ascontiguousarray(
            xs.transpose(2, 0, 1).reshape(KC, 128, NTOK)).astype(bf)
        hxT_c = np.ascontiguousarray(
            hx[bs].T.reshape(KC, 128, BL)).astype(bf)
        in_maps.append({
            "xT": xT_c, "hxT": hxT_c,
            "wx": wx_in, "wh": wh_in, "bias": bias_in,
            "ident": (np.eye(128, dtype=np.float32) * (SW * SH)
                      ).astype(ml_dtypes.bfloat16),
        })
    return in_maps


def run(inputs, trace=False):
    nc = _get_nc()
    in_maps = _prep_inputs(inputs)
    res = run_bass_kernel_spmd(nc, in_maps, core_ids=list(range(N_CORES)),
                               trace=trace)
    outputs = np.empty((T, B, 2 * H), np.float32)
    for c in range(N_CORES):
        o = res.results[c]["out"].astype(np.float32).reshape(2 * H, T, BL)
        outputs[:, c * BL:(c + 1) * BL, :] = o.transpose(1, 2, 0)
    return (outputs, outputs[-1].copy()), res.exec_time_ns


def kernel(**inputs):
    return run(inputs, trace=False)[0]


# revision 45
# speedup vs baseline: 1.0396x; 1.0194x over previous
"""Trainium2 Bass kernel for the CRU (complex recurrent unit) problem.

Math: the reference is a T=256-step scan
    pre_t  = x_t @ Wx_big + h_{t-1} @ Wh_big + bias          (B, 2048)
    gate   = sigmoid(pre gate cols), cand = tanh(pre cand cols)
    m      = softmax(x_t @ Wm_ih + h_{t-1} @ Wm_hh + bias_m)  (B, 512)
    h_t    = [m,m] * (gate*h_{t-1} + (1-gate)*cand)

Because m is a softmax over 512 features, |h| ~ 1e-2..1e-3 and the
recurrence is contractive with factor ~5e-3.  A Jacobi fixed-point
iteration over the whole sequence therefore converges in 2 sweeps
(measured: sweep1 rel_err 4.7e-3, sweep2 2.2e-5 in fp32), replacing 256
sequential weight-reload-bound steps with a few large batched matmuls.

Kernel layout (per core, batch-sharded 16 of 128, no collectives):
  feature-major: every tensor is (feature-chunk, 128, token) with
  token n = t*16 + b.  Sweep s computes, for all 4096 tokens at once,
    pre = W.T @ h_prevT + xa      (h_prevT read with a 16-column shift)
  and the gate/cand/softmax elementwise pipeline:
    tau = tanh(0.5*pre_g); u' = (1+tau) h + (1-tau) c
    h   = [e,e] * R * u'  with  e = exp(l - ln2), R = 1/(2*sum e)
  (the 0.5's from sigmoid(x)=0.5+0.5 tanh(x/2) are folded into the
  exp bias and the "2.0" weights of the column-sum matmul).
"""

import numpy as np
import ml_dtypes

import concourse.bass as bass
import concourse.bacc as bacc
import concourse.mybir as mybir
import concourse.tile as tile
from concourse.bass_utils import run_bass_kernel_spmd

BF16 = mybir.dt.bfloat16
F32 = mybir.dt.float32
FP8 = mybir.dt.float8e4
AF = mybir.ActivationFunctionType
ALU = mybir.AluOpType

N_CORES = 8
T, B, I, H = 256, 128, 512, 512
BL = B // N_CORES          # batch per core
NTOK = T * BL              # 4096 tokens per core
KC = 8                     # contraction chunks of 128 (K = 1024)
MC = 20                    # output-feature chunks of 128 (M = 2560)
GC, CC, EC = 8, 8, 4       # gate / cand / mod chunks within MC
TC = 8                     # token chunks of 512
TW = NTOK // TC            # 512 tokens per chunk
N_SWEEPS = 2
LN2 = float(np.log(2.0))
SW = 1024.0                # fp8 weight scale (h-sweep)
SH = 64.0                  # fp8 hidden-state scale

_NC_CACHE = {}


def _chunk_bcast(ap, nrep):
    """View a (128, TW) AP as (128, nrep, TW) with a stride-0 repeat dim."""
    return bass.AP(ap.tensor, ap.offset, [ap.ap[0], [0, nrep], ap.ap[1]])


def _build_phase(nc, tc, pools, w_sb, bias_sb, consts, first, rhs_dram, xa_dram,
                 dst_dram, dst_off, dst_fp8):
    """One Jacobi sweep.  first=True fuses the x-matmul with sweep 1
    (h_prev treated as 0 everywhere; the dropped hx@Wh term for t=0 is
    corrected by sweep 2, which reads hx from hg columns 0:BL).
    Non-first sweeps consume fp8 h (x SH) and fp8 weights (x SW) with
    DoubleRow pairing of adjacent k-chunks; descale rides the pre-STT."""
    rhsp, xap, actp, ewp, outp, pp, ppr, ppb = pools
    twos, ones_row, negln2, ident = consts
    n_acc = KC if first else KC // 2
    # non-first sweeps: psum holds 65536*pre (identity preload of xa is
    # scaled by SW*SH = 65536 to match the fp8 weight/state scales), so
    # the descale rides the ACT input affine for free.
    asc = 1.0 if first else 1.0 / (SW * SH)

    for tch in range(TC):
        tsl = slice(tch * TW, (tch + 1) * TW)
        # all 8 k-chunks in one DMA; doubles as h_prev for the elementwise
        hrt = rhsp.tile([128, KC, TW], BF16 if first else FP8, tag="rhs",
                        bufs=2, name="hrt")
        nc.sync.dma_start(hrt[:], rhs_dram[:, :, tsl].rearrange("k p n -> p k n"))
        if first:
            h_ew = hrt
        else:
            # descaled bf16 copy of h for the elementwise chain
            h_ew = rhsp.tile([128, KC, TW], BF16, tag="hbf", bufs=1, name="h_ew")
            nc.scalar.activation(h_ew[:], hrt[:], AF.Copy, scale=1.0 / SH)

        tau_t = actp.tile([128, GC, TW], BF16, tag="tau", name="tau_t")
        c_t = actp.tile([128, CC, TW], BF16, tag="cand", name="c_t")
        e_t = actp.tile([128, EC, TW], BF16, tag="e", name="e_t")

        # mod -> cand -> gate order: the softmax chain and the a/d ops can
        # then run while the gate matmuls still stream, shortening the
        # post-last-matmul tail to tanh -> p -> u -> hn.
        ms = (list(range(GC + CC, MC)) + list(range(GC, GC + CC))
              + list(range(GC)))
        # First tchunk runs k-outer over m-groups so the PE can start as
        # soon as weight chunk k=0 arrives instead of waiting for all 8.
        if tch == 0:
            m_plan = [ms[g:g + 4] for g in range(0, MC, 4)]
        else:
            m_plan = [[m] for m in ms]

        for mg in m_plan:
            pss = [pp.tile([128, TW], F32, tag="ps", name=f"ps{m}") for m in mg]
            xa_tiles = {}
            if not first:
                # preload psum with 65536*xa via a scaled-identity matmul
                for mi, m in enumerate(mg):
                    xa_sb = xap.tile([128, TW], BF16, tag="xa", bufs=6,
                                     name="xa_sb")
                    nc.sync.dma_start(xa_sb[:], xa_dram[m, :, tsl])
                    xa_tiles[m] = xa_sb
                for mi, m in enumerate(mg):
                    nc.tensor.matmul(pss[mi][:], ident[:], xa_tiles[m][:],
                                     start=True, stop=False)
            for k in range(n_acc):
                for mi, m in enumerate(mg):
                    if first:
                        nc.tensor.matmul(
                            pss[mi][:],
                            w_sb[k][:, m * 128:(m + 1) * 128],
                            hrt[:, k, :],
                            start=(k == 0),
                            stop=(k == n_acc - 1),
                        )
                    else:
                        nc.tensor.matmul(
                            pss[mi][:],
                            w_sb[k][:, :, m * 128:(m + 1) * 128],
                            hrt[:, 2 * k:2 * k + 2, :],
                            start=False,
                            stop=(k == n_acc - 1),
                            perf_mode=mybir.MatmulPerfMode.DoubleRow,
                        )
            for mi, m in enumerate(mg):
                ps = pss[mi]
                if first:
                    # pre = psum + bias; alternate ACT/DVE to balance engines
                    pre = xap.tile([128, TW], BF16, tag="xa", bufs=6,
                                   name="xa_sb")
                    if m % 2 == 0:
                        nc.scalar.add(pre[:], ps[:], bias_sb[:, m:m + 1])
                    else:
                        nc.vector.tensor_scalar_add(pre[:], ps[:],
                                                    bias_sb[:, m:m + 1])
                    nc.sync.dma_start(xa_dram[m, :, tsl], pre[:])
                    src = pre
                else:
                    src = ps  # activations descale the psum via `asc`
                # activations (tanh & exp share one ACT table set)
                if m < GC:
                    nc.scalar.activation(tau_t[:, m, :], src[:], AF.Tanh,
                                         scale=0.5 * asc)
                elif m < GC + CC:
                    nc.scalar.activation(c_t[:, m - GC, :], src[:], AF.Tanh,
                                         scale=asc)
                else:
                    nc.scalar.activation(e_t[:, m - GC - CC, :], src[:],
                                         AF.Exp, bias=negln2[:], scale=asc)

        # R = 1 / (2 * sum_feat e), broadcast to 128 partitions via PE
        s1 = ewp.tile([128, TW], BF16, tag="ssum", bufs=2, name="s1")
        s2 = ewp.tile([128, TW], BF16, tag="ssum2", bufs=2, name="s2")
        nc.vector.tensor_add(s1[:], e_t[:, 0, :], e_t[:, 1, :])
        nc.vector.tensor_add(s2[:], e_t[:, 2, :], e_t[:, 3, :])
        s3 = ewp.tile([128, TW], BF16, tag="ssum3", bufs=2, name="s3")
        nc.vector.tensor_add(s3[:], s1[:], s2[:])
        psr = ppr.tile([1, TW], F32, tag="psr", name="psr")
        nc.tensor.matmul(psr[:], twos[:], s3[:], start=True, stop=True)
        rv = ewp.tile([1, TW], F32, tag="rv", bufs=2, name="rv")
        nc.vector.reciprocal_approx_fast(rv[:], psr[:])
        psb = ppb.tile([128, TW], F32, tag="psb", name="psb")
        nc.tensor.matmul(psb[:], ones_row[:], rv[:], start=True, stop=True)
        rb = ewp.tile([128, TW], BF16, tag="rb", bufs=2, name="rb")
        nc.scalar.copy(rb[:], psb[:])

        # w = e * R  (one wide op, R repeated across the 4 chunks)
        w_t = ewp.tile([128, EC, TW], BF16, tag="wm", bufs=2, name="w_t")
        nc.vector.tensor_tensor(w_t[:], e_t[:], _chunk_bcast(rb[:], EC), ALU.mult)

        # u' = (1+tau) h + (1-tau) c  as wide plain-TT ops (2x DVE mode)
        if first:
            p0 = ewp.tile([128, KC, TW], BF16, tag="up", bufs=1, name="p0")
            nc.vector.tensor_tensor(p0[:], tau_t[:], c_t[:], ALU.mult)
            ut = ewp.tile([128, KC, TW], BF16, tag="uu", bufs=1, name="ut")
            nc.vector.tensor_tensor(ut[:], c_t[:], p0[:], ALU.subtract)
        else:
            at = ewp.tile([128, KC, TW], BF16, tag="ua", bufs=1, name="at")
            nc.vector.tensor_tensor(at[:], h_ew[:], c_t[:], ALU.add)
            dt_ = ewp.tile([128, KC, TW], BF16, tag="ud", bufs=1, name="dt_")
            nc.vector.tensor_tensor(dt_[:], h_ew[:], c_t[:], ALU.subtract)
            p0 = ewp.tile([128, KC, TW], BF16, tag="up", bufs=1, name="p0")
            nc.vector.tensor_tensor(p0[:], tau_t[:], dt_[:], ALU.mult)
            ut = ewp.tile([128, KC, TW], BF16, tag="uu", bufs=1, name="ut")
            nc.vector.tensor_tensor(ut[:], at[:], p0[:], ALU.add)

        # h_new = [w,w] * u'   (two half-wide ops reusing w_t)
        hn = outp.tile([128, KC, TW], BF16, tag="hn", bufs=2, name="hn")
        nc.vector.tensor_tensor(hn[:, 0:EC, :], w_t[:], ut[:, 0:EC, :], ALU.mult)
        nc.vector.tensor_tensor(hn[:, EC:KC, :], w_t[:], ut[:, EC:KC, :], ALU.mult)
        if dst_fp8:
            hn8 = outp.tile([128, KC, TW], FP8, tag="hn8", bufs=1, name="hn8")
            nc.scalar.activation(hn8[:], hn[:], AF.Copy, scale=SH)
            hn = hn8
        nc.sync.dma_start(
            dst_dram[:, :, dst_off + tch * TW: dst_off + (tch + 1) * TW]
            .rearrange("k p n -> p k n"),
            hn[:])


def _build_nc():
    nc = bacc.Bacc("TRN2", target_bir_lowering=False)

    xT = nc.dram_tensor("xT", (KC, 128, NTOK), BF16, kind="ExternalInput")
    hxT = nc.dram_tensor("hxT", (KC, 128, BL), BF16, kind="ExternalInput")
    wx = nc.dram_tensor("wx", (KC, 128, MC * 128), BF16, kind="ExternalInput")
    wh = nc.dram_tensor("wh", (KC // 2, 128, 2, MC * 128), FP8,
                        kind="ExternalInput")
    bias = nc.dram_tensor("bias", (128, MC), F32, kind="ExternalInput")
    ident_d = nc.dram_tensor("ident", (128, 128), BF16, kind="ExternalInput")
    out = nc.dram_tensor("out", (KC, 128, NTOK), BF16, kind="ExternalOutput")

    xa_d = nc.dram_tensor("xa_scratch", (MC, 128, NTOK), BF16)
    hg = [nc.dram_tensor(f"hg_{i}", (KC, 128, BL + NTOK), FP8)
          for i in range(N_SWEEPS - 1)]

    with tile.TileContext(nc) as tc:
        with (
            tc.tile_pool(name="weights", bufs=2) as wp,
            tc.tile_pool(name="const", bufs=1) as cp,
            tc.tile_pool(name="rhs", bufs=2) as rhsp,
            tc.tile_pool(name="xa", bufs=6) as xap,
            tc.tile_pool(name="acts", bufs=2) as actp,
            tc.tile_pool(name="ew", bufs=2) as ewp,
            tc.tile_pool(name="hout", bufs=2) as outp,
            tc.tile_pool(name="ps", bufs=6, space=bass.MemorySpace.PSUM) as pp,
            tc.tile_pool(name="psr", bufs=1, space=bass.MemorySpace.PSUM) as ppr,
            tc.tile_pool(name="psb", bufs=1, space=bass.MemorySpace.PSUM) as ppb,
        ):
            pools = (rhsp, xap, actp, ewp, outp, pp, ppr, ppb)

            bias_sb = cp.tile([128, MC], F32, tag="bias", name="bias_sb")
            nc.sync.dma_start(bias_sb[:], bias[:])
            twos = cp.tile([128, 1], BF16, tag="twos", name="twos")
            nc.vector.memset(twos[:], 2.0)
            ones_row = cp.tile([1, 128], F32, tag="ones_row", name="ones_row")
            nc.vector.memset(ones_row[:], 1.0)
            negln2 = cp.tile([128, 1], F32, tag="negln2", name="negln2")
            nc.vector.memset(negln2[:], -LN2)
            ident = cp.tile([128, 128], BF16, tag="ident", name="ident")
            nc.sync.dma_start(ident[:], ident_d[:])
            consts = (twos, ones_row, negln2, ident)

            # hx -> first BL columns of hg_0 (via SBUF, scaled to fp8 state)
            hx_sb = cp.tile([128, KC, BL], BF16, tag="hx", name="hx_sb")
            nc.sync.dma_start(hx_sb[:], hxT.rearrange("k p b -> p k b"))
            hx8 = cp.tile([128, KC, BL], FP8, tag="hx8", name="hx8")
            nc.scalar.activation(hx8[:], hx_sb[:], AF.Copy, scale=SH)
            nc.sync.dma_start(hg[0][:, :, 0:BL].rearrange("k p b -> p k b"),
                              hx8[:])

            # sweep 1 (fused x-matmul), writes xa + hg_0 shifted by BL
            wx_sb = [wp.tile([128, MC * 128], BF16, tag=f"w{k}", bufs=1,
                             name=f"wx{k}") for k in range(KC)]
            for k in range(KC):
                nc.sync.dma_start(wx_sb[k][:], wx[k])
            _build_phase(nc, tc, pools, wx_sb, bias_sb, consts, True,
                         xT, xa_d, hg[0], BL, True)

            # sweeps 2..N (fp8 DoubleRow: 4 tiles pairing adjacent k-chunks)
            wh_sb = [wp.tile([128, 2, MC * 128], FP8, tag=f"w{j}", bufs=1,
                             name=f"wh{j}") for j in range(KC // 2)]
            for j in range(KC // 2):
                nc.sync.dma_start(wh_sb[j][:], wh[j])
            for s in range(1, N_SWEEPS):
                last = s == N_SWEEPS - 1
                dst = out if last else hg[s]
                _build_phase(nc, tc, pools, wh_sb, bias_sb, consts, False,
                             hg[s - 1], xa_d, dst, 0 if last else BL, not last)

    nc.compile()
    return nc


def _get_nc():
    if "nc" not in _NC_CACHE:
        _NC_CACHE["nc"] = _build_nc()
    return _NC_CACHE["nc"]


def _prep_inputs(inputs):
    f = lambda k: np.asarray(inputs[k], np.float32)
    x, hx = f("x"), f("hx")
    Wr_ih, Wi_ih = f("w_ih_real"), f("w_ih_imag")
    Wr_hh, Wi_hh = f("w_hh_real"), f("w_hh_imag")
    b_ih, b_hh = f("b_ih"), f("b_hh")
    Wm_ih, Wm_hh = f("w_ih_mod"), f("w_hh_mod")
    bm_ih, bm_hh = f("b_ih_mod"), f("b_hh_mod")

    def big(Wr, Wi):
        return np.block([[Wr, Wi], [-Wi, Wr]])

    perm = np.concatenate([
        np.arange(0, 512), np.arange(1024, 1536),       # gate: [gr | gi]
        np.arange(512, 1024), np.arange(1536, 2048),    # cand: [cr | ci]
    ])
    W_x = np.concatenate([big(Wr_ih, Wi_ih)[:, perm], Wm_ih], axis=1)
    W_h = np.concatenate([big(Wr_hh, Wi_hh)[:, perm], Wm_hh], axis=1)
    bias_all = np.concatenate([(b_ih + b_hh)[perm], bm_ih + bm_hh])

    bf = ml_dtypes.bfloat16
    wx_in = np.ascontiguousarray(W_x.reshape(KC, 128, MC * 128)).astype(bf)
    # fp8 h-weights: scale by SW, pair adjacent k-chunks for DoubleRow:
    # [pair j, partition p, ko, m] = W_h[(2j+ko)*128 + p, m] * SW
    wh8 = np.clip(W_h * SW, -448.0, 448.0)
    wh_in = np.ascontiguousarray(
        wh8.reshape(KC // 2, 2, 128, MC * 128).transpose(0, 2, 1, 3)
    ).astype(ml_dtypes.float8_e4m3)
    bias_in = np.ascontiguousarray(bias_all.reshape(MC, 128).T).astype(np.float32)

    in_maps = []
    for c in range(N_CORES):
        bs = slice(c * BL, (c + 1) * BL)
        xs = x[:, bs, :]                                   # (T, BL, 1024)
        xT_c = np.ascontiguousarray(
            xs.transpose(2, 0, 1).reshape(KC, 128, NTOK)).astype(bf)
        hxT_c = np.ascontiguousarray(
            hx[bs].T.reshape(KC, 128, BL)).astype(bf)
        in_maps.append({
            "xT": xT_c, "hxT": hxT_c,
            "wx": wx_in, "wh": wh_in, "bias": bias_in,
            "ident": (np.eye(128, dtype=np.float32) * (SW * SH)
                      ).astype(ml_dtypes.bfloat16),
        })
    return in_maps


def run(inputs, trace=False):
    nc = _get_nc()
    in_maps = _prep_inputs(inputs)
    res = run_bass_kernel_spmd(nc, in_maps, core_ids=list(range(N_CORES)),
                               trace=trace)
    outputs = np.empty((T, B, 2 * H), np.float32)
    for c in range(N_CORES):
        o = res.results[c]["out"].astype(np.float32).reshape(2 * H, T, BL)
        outputs[:, c * BL:(c + 1) * BL, :] = o.transpose(1, 2, 0)
    return (outputs, outputs[-1].copy()), res.exec_time_ns


def kernel(**inputs):
    return run(inputs, trace=False)[0]
